# revision 11
# baseline (speedup 1.0000x reference)
"""Trainium2 Bass kernel for nn_AttnProcessor (B=2, S=4096, D=512, H=8).

Sharding: 8 cores; core c handles batch b=c//4, head-pair hp=c%4 (heads 2hp, 2hp+1).
Each core computes q/k/v projections for its 128-column weight slice, full SxS
attention for its two heads (scores^T orientation, no-max softmax with the
denominator obtained via a ones-column appended to V), and a partial output
projection. Host sums the 4 partials per batch and adds bo.

All matmuls run in float32r (TF32-like fast mode, ~1.5e-4 rel err).
"""
import os
os.environ.setdefault("JAX_PLATFORMS", "")

import numpy as np

import concourse.bass as bass
import concourse.tile as tile
from concourse import bacc, mybir
from concourse import bass_utils
from concourse.masks import make_identity

F32 = mybir.dt.float32
F32R = mybir.dt.float32r
AF = mybir.ActivationFunctionType

B, S, D, H = 2, 4096, 512, 8
HD = D // H            # 64
NCORES = 8
ST = S // 128          # 32 seq tiles
KT = D // 128          # 4 contraction tiles for projections
MCH = 512              # m chunk (q rows per attention chunk)
NCH = S // MCH         # 8 chunks
NT = S // 128          # 32 key tiles
SCALE = 1.0 / np.sqrt(HD)


def _build_body(nc, tc, X, WQ, WK, WV, BQ, BK, BV, WO, OUT, RSCR, ONES):
    with (
        tc.tile_pool(name="cons", bufs=1) as cons,
        tc.tile_pool(name="big", bufs=1) as big,
    ):
        ident = cons.tile([128, 128], F32)
        make_identity(nc, ident[:])

        # weight stationaries (f32r via bitcast DMA)
        wq = cons.tile([128, KT, 128], F32R, tag="wq")
        wk = cons.tile([128, KT, 128], F32R, tag="wk")
        wv = cons.tile([128, KT, 128], F32R, tag="wv")
        for w_sb, w_dram in ((wq, WQ), (wk, WK), (wv, WV)):
            nc.sync.dma_start(
                out=w_sb[:],
                in_=w_dram[:].rearrange("(kt p) m -> p kt m", p=128).bitcast(F32R),
            )
        wo_sb = [cons.tile([64, MCH], F32R, tag=f"wo{h}", name=f"wo{h}") for h in range(2)]
        for h in range(2):
            nc.sync.dma_start(out=wo_sb[h][:], in_=WO[h * 64:(h + 1) * 64, :].bitcast(F32R))
        bq = cons.tile([128, 1], F32, tag="bq")
        bk = cons.tile([128, 1], F32, tag="bk")
        bv = cons.tile([128, 1], F32, tag="bv")
        for b_sb, b_dram in ((bq, BQ), (bk, BK), (bv, BV)):
            nc.sync.dma_start(out=b_sb[:], in_=b_dram[:])

        # persistent big tensors
        xT = big.tile([128, KT, S], F32R, tag="xT")       # x^T  (d on partitions)
        qT = big.tile([128, S], F32R, tag="qT")           # heads-stacked q^T
        kTt = big.tile([128, S], F32R, tag="kT")
        v1 = [big.tile([128, NT, HD + 1], F32R, tag=f"v1_{h}", name=f"v1_{h}") for h in range(2)]
        oT = [big.tile([64, S], F32R, tag=f"oT{h}", name=f"oT{h}") for h in range(2)]

        # ---------------- Phase A: x^T via PE transposes -------------------
        with (
            tc.tile_pool(name="xs_pool", bufs=3) as xs_pool,
            tc.tile_pool(name="vT_pool", bufs=1) as vT_pool,
            tc.tile_pool(name="psA", bufs=4, space="PSUM") as psA,
            tc.tile_pool(name="psB", bufs=3, space="PSUM") as psB,
        ):
            for st in range(ST):
                xs = xs_pool.tile([128, D], F32, tag="xs")
                nc.sync.dma_start(out=xs[:], in_=X[st * 128:(st + 1) * 128, :])
                for kt in range(KT):
                    pt = psA.tile([128, 128], F32, tag="tr")
                    nc.tensor.transpose(pt[:], xs[:, kt * 128:(kt + 1) * 128], ident[:])
                    nc.vector.tensor_copy(xT[:, kt, st * 128:(st + 1) * 128], pt[:])

            # ------------- Phase B: projections q^T, k^T, v^T --------------
            vT = vT_pool.tile([128, S], F32)
            for w_sb, b_sb, dest in ((wq, bq, qT), (wk, bk, kTt), (wv, bv, vT)):
                for ch in range(NCH):
                    ps = psB.tile([128, MCH], F32, tag="proj")
                    for kt in range(KT):
                        nc.tensor.matmul(
                            ps[:], w_sb[:, kt, :], xT[:, kt, ch * MCH:(ch + 1) * MCH],
                            start=(kt == 0), stop=(kt == KT - 1),
                        )
                    nc.vector.tensor_scalar_add(
                        dest[:, ch * MCH:(ch + 1) * MCH], ps[:], b_sb[:]
                    )

            # ------------- Phase C: natural v (+ ones column) --------------
            for h in range(2):
                nc.gpsimd.dma_start(
                    out=v1[h][:, :, HD:HD + 1],
                    in_=ONES[:].bitcast(F32R).unsqueeze(2),
                )
                for nt in range(NT):
                    pt = psA.tile([128, HD], F32, tag="tr", name="pt")
                    nc.tensor.transpose(
                        pt[:],
                        vT[h * 64:(h + 1) * 64, nt * 128:(nt + 1) * 128],
                        ident[h * 64:(h + 1) * 64, h * 64:(h + 1) * 64],
                    )
                    nc.vector.tensor_copy(v1[h][:, nt, 0:HD], pt[:])

        # ---------------- Phase D: attention -------------------------------
        with (
            tc.tile_pool(name="esP", bufs=3) as esP,
            tc.tile_pool(name="nrm", bufs=4) as nrm,
            tc.tile_pool(name="psS", bufs=2, space="PSUM") as psS_pool,
            tc.tile_pool(name="psO", bufs=4, space="PSUM") as psO_pool,
        ):
            for ch in range(NCH):
                mslice = slice(ch * MCH, (ch + 1) * MCH)
                psO = [psO_pool.tile([HD + 1, MCH], F32, tag="psO", name="psO") for _ in range(2)]
                for npair in range(NT // 2):
                    psS = [psS_pool.tile([128, 1024], F32, tag="psS", name="psS") for _ in range(2)]
                    for j in range(2):
                        nt = npair * 2 + j
                        for h in range(2):
                            nc.tensor.matmul(
                                psS[h][:, j * 512:(j + 1) * 512],
                                kTt[h * 64:(h + 1) * 64, nt * 128:(nt + 1) * 128],
                                qT[h * 64:(h + 1) * 64, mslice],
                                start=True, stop=True,
                            )
                    for h in range(2):
                        eS = esP.tile([128, 1024], F32R, tag="eS")
                        nc.scalar.activation(eS[:], psS[h][:], AF.Exp, scale=float(SCALE))
                        for j in range(2):
                            nt = npair * 2 + j
                            nc.tensor.matmul(
                                psO[h][:], v1[h][:, nt, :], eS[:, j * 512:(j + 1) * 512],
                                start=(nt == 0), stop=(nt == NT - 1),
                                skip_group_check=True,
                            )
                # normalize: oT[h] = psO[rows 0:64] * (1 / psO[row 64]) (bcast)
                for h in range(2):
                    rd = nrm.tile([1, MCH], F32, tag="rd")
                    nc.vector.reciprocal(rd[:], psO[h][HD:HD + 1, :])
                    nc.sync.dma_start(out=RSCR[ch, h, :].unsqueeze(0), in_=rd[:])
                    rb = nrm.tile([64, MCH], F32, tag="rb")
                    src = RSCR[ch, h, :]
                    bcast = bass.AP(tensor=src.tensor, offset=src.offset,
                                    ap=[[0, 64]] + list(src.ap))
                    nc.gpsimd.dma_start(out=rb[:], in_=bcast)
                    nc.vector.tensor_tensor(
                        out=oT[h][:, mslice], in0=psO[h][0:HD, :], in1=rb[:],
                        op=mybir.AluOpType.mult,
                    )

        # ---------------- Phase E: output projection -----------------------
        with (
            tc.tile_pool(name="outP", bufs=3) as outP,
            tc.tile_pool(name="psE", bufs=3, space="PSUM") as psE,
        ):
            for mt in range(ST):
                ps = psE.tile([128, MCH], F32, tag="psE")
                for h in range(2):
                    nc.tensor.matmul(
                        ps[:], oT[h][:, mt * 128:(mt + 1) * 128], wo_sb[h][:],
                        start=(h == 0), stop=(h == 1),
                    )
                ob = outP.tile([128, MCH], F32, tag="ob")
                nc.vector.tensor_copy(ob[:], ps[:])
                nc.sync.dma_start(out=OUT[mt * 128:(mt + 1) * 128, :], in_=ob[:])


def build_kernel():
    nc = bacc.Bacc("TRN2", target_bir_lowering=False)
    with tile.TileContext(nc) as tc:
        with tc.tile_pool(name="dram", bufs=1, space="DRAM") as dram:
            X = dram.tile([S, D], F32, kind="ExternalInput")
            WQ = dram.tile([D, 128], F32, kind="ExternalInput")
            WK = dram.tile([D, 128], F32, kind="ExternalInput")
            WV = dram.tile([D, 128], F32, kind="ExternalInput")
            BQ = dram.tile([128, 1], F32, kind="ExternalInput")
            BK = dram.tile([128, 1], F32, kind="ExternalInput")
            BV = dram.tile([128, 1], F32, kind="ExternalInput")
            WO = dram.tile([128, D], F32, kind="ExternalInput")
            OUT = dram.tile([S, D], F32, kind="ExternalOutput")
            RSCR = dram.tile([NCH, 2, MCH], F32, kind="Internal")
            ONES = dram.tile([128, NT], F32, kind="ExternalInput")
            _build_body(nc, tc, X, WQ, WK, WV, BQ, BK, BV, WO, OUT, RSCR, ONES)
    nc.compile()
    names = dict(X=X.name, WQ=WQ.name, WK=WK.name, WV=WV.name, BQ=BQ.name,
                 BK=BK.name, BV=BV.name, WO=WO.name, OUT=OUT.name, ONES=ONES.name)
    return nc, names


_CACHE = {}


def _get_kernel():
    if "k" not in _CACHE:
        _CACHE["k"] = build_kernel()
    return _CACHE["k"]


def make_in_maps(hidden_states, Wq, bq, Wk, bk, Wv, bv, Wo, bo, names):
    x = np.asarray(hidden_states, dtype=np.float32).reshape(B, S, D)
    in_maps = []
    for c in range(NCORES):
        b, hp = c // 4, c % 4
        sl = slice(hp * 128, (hp + 1) * 128)
        in_maps.append({
            names["X"]: np.ascontiguousarray(x[b]),
            names["WQ"]: np.ascontiguousarray(np.asarray(Wq, np.float32)[:, sl]),
            names["WK"]: np.ascontiguousarray(np.asarray(Wk, np.float32)[:, sl]),
            names["WV"]: np.ascontiguousarray(np.asarray(Wv, np.float32)[:, sl]),
            names["BQ"]: np.ascontiguousarray(np.asarray(bq, np.float32)[sl].reshape(128, 1)),
            names["BK"]: np.ascontiguousarray(np.asarray(bk, np.float32)[sl].reshape(128, 1)),
            names["BV"]: np.ascontiguousarray(np.asarray(bv, np.float32)[sl].reshape(128, 1)),
            names["WO"]: np.ascontiguousarray(np.asarray(Wo, np.float32)[sl, :]),
            names["ONES"]: np.ones((128, NT), dtype=np.float32),
        })
    return in_maps


def combine_outputs(results, names, bo):
    out = np.zeros((B, S, D), dtype=np.float64)
    for c in range(NCORES):
        out[c // 4] += results[c][names["OUT"]].astype(np.float64)
    out += np.asarray(bo, np.float64)
    return out.astype(np.float32)


def run(hidden_states, Wq, bq, Wk, bk, Wv, bv, Wo, bo, trace=False, **spmd_kwargs):
    nc, names = _get_kernel()
    in_maps = make_in_maps(hidden_states, Wq, bq, Wk, bk, Wv, bv, Wo, bo, names)
    res = bass_utils.run_bass_kernel_spmd(
        nc, in_maps, core_ids=list(range(NCORES)), trace=trace, **spmd_kwargs
    )
    out = combine_outputs(res.results, names, bo)
    return out, res


def kernel(hidden_states, Wq, bq, Wk, bk, Wv, bv, Wo, bo):
    out, _ = run(hidden_states, Wq, bq, Wk, bk, Wv, bv, Wo, bo, trace=False)
    return out


# revision 13
# speedup vs baseline: 1.1050x; 1.1050x over previous
"""Trainium2 Bass kernel for nn_AttnProcessor (B=2, S=4096, D=512, H=8).

Sharding: 8 cores; core c handles batch b=c//4, head-pair hp=c%4 (heads 2hp, 2hp+1).
Each core computes q/k/v projections for its 128-column weight slice, full SxS
attention for its two heads (scores^T orientation, no-max softmax with the
denominator obtained via a ones-column appended to V), and a partial output
projection. Host sums the 4 partials per batch and adds bo.

All matmuls run in float32r (TF32-like fast mode, ~1.5e-4 rel err).
"""
import os
os.environ.setdefault("JAX_PLATFORMS", "")

import numpy as np

import concourse.bass as bass
import concourse.tile as tile
from concourse import bacc, mybir
from concourse import bass_utils
from concourse.masks import make_identity

F32 = mybir.dt.float32
F32R = mybir.dt.float32r
BF16 = mybir.dt.bfloat16
AF = mybir.ActivationFunctionType

B, S, D, H = 2, 4096, 512, 8
HD = D // H            # 64
NCORES = 8
ST = S // 128          # 32 seq tiles
KT = D // 128          # 4 contraction tiles for projections
MCH = 512              # m chunk (q rows per attention chunk)
NCH = S // MCH         # 8 chunks
NT = S // 128          # 32 key tiles
SCALE = 1.0 / np.sqrt(HD)


def _build_body(nc, tc, X, WQ, WK, WV, BQ, BK, BV, WO, OUT, RSCR, ONES):
    with (
        tc.tile_pool(name="cons", bufs=1) as cons,
        tc.tile_pool(name="big", bufs=1) as big,
    ):
        ident = cons.tile([128, 128], F32)
        make_identity(nc, ident[:])

        # weight stationaries (f32r via bitcast DMA)
        wq = cons.tile([128, KT, 128], F32R, tag="wq")
        wk = cons.tile([128, KT, 128], F32R, tag="wk")
        wv = cons.tile([128, KT, 128], F32R, tag="wv")
        for w_sb, w_dram in ((wq, WQ), (wk, WK), (wv, WV)):
            nc.sync.dma_start(
                out=w_sb[:],
                in_=w_dram[:].rearrange("(kt p) m -> p kt m", p=128).bitcast(F32R),
            )
        wo_sb = [cons.tile([64, MCH], BF16, tag=f"wo{h}", name=f"wo{h}") for h in range(2)]
        for h in range(2):
            nc.gpsimd.dma_start(out=wo_sb[h][:], in_=WO[h * 64:(h + 1) * 64, :])
        bq = cons.tile([128, 1], F32, tag="bq")
        bk = cons.tile([128, 1], F32, tag="bk")
        bv = cons.tile([128, 1], F32, tag="bv")
        for b_sb, b_dram in ((bq, BQ), (bk, BK), (bv, BV)):
            nc.sync.dma_start(out=b_sb[:], in_=b_dram[:])

        # persistent big tensors
        xT = big.tile([128, KT, S], F32R, tag="xT")       # x^T  (d on partitions)
        qT = big.tile([128, S], BF16, tag="qT")           # heads-stacked q^T
        kTt = big.tile([128, S], BF16, tag="kT")
        v1 = [big.tile([128, NT, HD + 1], BF16, tag=f"v1_{h}", name=f"v1_{h}") for h in range(2)]
        oT = [big.tile([64, S], BF16, tag=f"oT{h}", name=f"oT{h}") for h in range(2)]

        # ---------------- Phase A: x^T via PE transposes -------------------
        with (
            tc.tile_pool(name="xs_pool", bufs=3) as xs_pool,
            tc.tile_pool(name="vT_pool", bufs=1) as vT_pool,
            tc.tile_pool(name="psA", bufs=4, space="PSUM") as psA,
            tc.tile_pool(name="psB", bufs=3, space="PSUM") as psB,
        ):
            for st in range(ST):
                xs = xs_pool.tile([128, D], F32, tag="xs")
                nc.sync.dma_start(out=xs[:], in_=X[st * 128:(st + 1) * 128, :])
                for kt in range(KT):
                    pt = psA.tile([128, 128], F32, tag="tr")
                    nc.tensor.transpose(pt[:], xs[:, kt * 128:(kt + 1) * 128], ident[:])
                    nc.vector.tensor_copy(xT[:, kt, st * 128:(st + 1) * 128], pt[:])

            # ------------- Phase B: projections q^T, k^T, v^T --------------
            vT = vT_pool.tile([128, S], F32)
            for w_sb, b_sb, dest in ((wq, bq, qT), (wk, bk, kTt), (wv, bv, vT)):
                for ch in range(NCH):
                    ps = psB.tile([128, MCH], F32, tag="proj")
                    for kt in range(KT):
                        nc.tensor.matmul(
                            ps[:], w_sb[:, kt, :], xT[:, kt, ch * MCH:(ch + 1) * MCH],
                            start=(kt == 0), stop=(kt == KT - 1),
                        )
                    nc.vector.tensor_scalar_add(
                        dest[:, ch * MCH:(ch + 1) * MCH], ps[:], b_sb[:]
                    )

            # ------------- Phase C: natural v (+ ones column) --------------
            for h in range(2):
                nc.gpsimd.dma_start(
                    out=v1[h][:, :, HD:HD + 1],
                    in_=ONES[:].unsqueeze(2),
                )
                for nt in range(NT):
                    pt = psA.tile([128, HD], F32, tag="tr", name="pt")
                    nc.tensor.transpose(
                        pt[:],
                        vT[h * 64:(h + 1) * 64, nt * 128:(nt + 1) * 128],
                        ident[h * 64:(h + 1) * 64, h * 64:(h + 1) * 64],
                    )
                    nc.vector.tensor_copy(v1[h][:, nt, 0:HD], pt[:])

        # ---------------- Phase D: attention -------------------------------
        with (
            tc.tile_pool(name="esP", bufs=3) as esP,
            tc.tile_pool(name="nrm", bufs=4) as nrm,
            tc.tile_pool(name="psS", bufs=2, space="PSUM") as psS_pool,
            tc.tile_pool(name="psO", bufs=4, space="PSUM") as psO_pool,
        ):
            for ch in range(NCH):
                mslice = slice(ch * MCH, (ch + 1) * MCH)
                psO = [psO_pool.tile([HD + 1, MCH], F32, tag="psO", name="psO") for _ in range(2)]
                for npair in range(NT // 2):
                    psS = [psS_pool.tile([128, 1024], F32, tag="psS", name="psS") for _ in range(2)]
                    for j in range(2):
                        nt = npair * 2 + j
                        for h in range(2):
                            nc.tensor.matmul(
                                psS[h][:, j * 512:(j + 1) * 512],
                                kTt[h * 64:(h + 1) * 64, nt * 128:(nt + 1) * 128],
                                qT[h * 64:(h + 1) * 64, mslice],
                                start=True, stop=True,
                            )
                    for h in range(2):
                        eS = esP.tile([128, 1024], BF16, tag="eS")
                        nc.scalar.activation(eS[:], psS[h][:], AF.Exp, scale=float(SCALE))
                        for j in range(2):
                            nt = npair * 2 + j
                            nc.tensor.matmul(
                                psO[h][:], v1[h][:, nt, :], eS[:, j * 512:(j + 1) * 512],
                                start=(nt == 0), stop=(nt == NT - 1),
                                skip_group_check=True,
                            )
                # normalize: oT[h] = psO[rows 0:64] * (1 / psO[row 64]) (bcast)
                for h in range(2):
                    rd = nrm.tile([1, MCH], F32, tag="rd")
                    nc.vector.reciprocal(rd[:], psO[h][HD:HD + 1, :])
                    nc.sync.dma_start(out=RSCR[ch, h, :].unsqueeze(0), in_=rd[:])
                    rb = nrm.tile([64, MCH], F32, tag="rb")
                    src = RSCR[ch, h, :]
                    bcast = bass.AP(tensor=src.tensor, offset=src.offset,
                                    ap=[[0, 64]] + list(src.ap))
                    nc.gpsimd.dma_start(out=rb[:], in_=bcast)
                    nc.vector.tensor_tensor(
                        out=oT[h][:, mslice], in0=psO[h][0:HD, :], in1=rb[:],
                        op=mybir.AluOpType.mult,
                    )

        # ---------------- Phase E: output projection -----------------------
        with (
            tc.tile_pool(name="outP", bufs=3) as outP,
            tc.tile_pool(name="psE", bufs=3, space="PSUM") as psE,
        ):
            for mt in range(ST):
                ps = psE.tile([128, MCH], F32, tag="psE")
                for h in range(2):
                    nc.tensor.matmul(
                        ps[:], oT[h][:, mt * 128:(mt + 1) * 128], wo_sb[h][:],
                        start=(h == 0), stop=(h == 1),
                    )
                ob = outP.tile([128, MCH], F32, tag="ob")
                nc.vector.tensor_copy(ob[:], ps[:])
                nc.sync.dma_start(out=OUT[mt * 128:(mt + 1) * 128, :], in_=ob[:])


def build_kernel():
    nc = bacc.Bacc("TRN2", target_bir_lowering=False)
    with tile.TileContext(nc) as tc:
        with tc.tile_pool(name="dram", bufs=1, space="DRAM") as dram:
            X = dram.tile([S, D], F32, kind="ExternalInput")
            WQ = dram.tile([D, 128], F32, kind="ExternalInput")
            WK = dram.tile([D, 128], F32, kind="ExternalInput")
            WV = dram.tile([D, 128], F32, kind="ExternalInput")
            BQ = dram.tile([128, 1], F32, kind="ExternalInput")
            BK = dram.tile([128, 1], F32, kind="ExternalInput")
            BV = dram.tile([128, 1], F32, kind="ExternalInput")
            WO = dram.tile([128, D], F32, kind="ExternalInput")
            OUT = dram.tile([S, D], F32, kind="ExternalOutput")
            RSCR = dram.tile([NCH, 2, MCH], F32, kind="Internal")
            ONES = dram.tile([128, NT], F32, kind="ExternalInput")
            _build_body(nc, tc, X, WQ, WK, WV, BQ, BK, BV, WO, OUT, RSCR, ONES)
    nc.compile()
    names = dict(X=X.name, WQ=WQ.name, WK=WK.name, WV=WV.name, BQ=BQ.name,
                 BK=BK.name, BV=BV.name, WO=WO.name, OUT=OUT.name, ONES=ONES.name)
    return nc, names


_CACHE = {}


def _get_kernel():
    if "k" not in _CACHE:
        _CACHE["k"] = build_kernel()
    return _CACHE["k"]


def make_in_maps(hidden_states, Wq, bq, Wk, bk, Wv, bv, Wo, bo, names):
    x = np.asarray(hidden_states, dtype=np.float32).reshape(B, S, D)
    in_maps = []
    for c in range(NCORES):
        b, hp = c // 4, c % 4
        sl = slice(hp * 128, (hp + 1) * 128)
        in_maps.append({
            names["X"]: np.ascontiguousarray(x[b]),
            names["WQ"]: np.ascontiguousarray(np.asarray(Wq, np.float32)[:, sl]),
            names["WK"]: np.ascontiguousarray(np.asarray(Wk, np.float32)[:, sl]),
            names["WV"]: np.ascontiguousarray(np.asarray(Wv, np.float32)[:, sl]),
            names["BQ"]: np.ascontiguousarray(np.asarray(bq, np.float32)[sl].reshape(128, 1)),
            names["BK"]: np.ascontiguousarray(np.asarray(bk, np.float32)[sl].reshape(128, 1)),
            names["BV"]: np.ascontiguousarray(np.asarray(bv, np.float32)[sl].reshape(128, 1)),
            names["WO"]: np.ascontiguousarray(np.asarray(Wo, np.float32)[sl, :]),
            names["ONES"]: np.ones((128, NT), dtype=np.float32),
        })
    return in_maps


def combine_outputs(results, names, bo):
    out = np.zeros((B, S, D), dtype=np.float64)
    for c in range(NCORES):
        out[c // 4] += results[c][names["OUT"]].astype(np.float64)
    out += np.asarray(bo, np.float64)
    return out.astype(np.float32)


def run(hidden_states, Wq, bq, Wk, bk, Wv, bv, Wo, bo, trace=False, **spmd_kwargs):
    nc, names = _get_kernel()
    in_maps = make_in_maps(hidden_states, Wq, bq, Wk, bk, Wv, bv, Wo, bo, names)
    res = bass_utils.run_bass_kernel_spmd(
        nc, in_maps, core_ids=list(range(NCORES)), trace=trace, **spmd_kwargs
    )
    out = combine_outputs(res.results, names, bo)
    return out, res


def kernel(hidden_states, Wq, bq, Wk, bk, Wv, bv, Wo, bo):
    out, _ = run(hidden_states, Wq, bq, Wk, bk, Wv, bv, Wo, bo, trace=False)
    return out


# revision 15
# speedup vs baseline: 1.4430x; 1.3059x over previous
"""Trainium2 Bass kernel for nn_AttnProcessor (B=2, S=4096, D=512, H=8).

Sharding: 8 cores; core c handles batch b=c//4, head-pair hp=c%4 (heads 2hp, 2hp+1).
Each core computes q/k/v projections for its 128-column weight slice, full SxS
attention for its two heads (scores^T orientation, no-max softmax with the
denominator obtained via a ones-column appended to V), and a partial output
projection. Host sums the 4 partials per batch and adds bo.

Projections run in float32r (TF32-like, ~1.5e-4); attention matmuls in bf16.
Phase D software-pipelines PV one step behind scores/exp so the tensor engine
never stalls on the scalar engine (keeps the HAM clock-gate at full rate).
"""
import os
os.environ.setdefault("JAX_PLATFORMS", "")

import numpy as np

import concourse.bass as bass
import concourse.tile as tile
from concourse import bacc, mybir
from concourse import bass_utils

F32 = mybir.dt.float32
F32R = mybir.dt.float32r
BF16 = mybir.dt.bfloat16
AF = mybir.ActivationFunctionType

B, S, D, H = 2, 4096, 512, 8
HD = D // H            # 64
NCORES = 8
ST = S // 128          # 32 seq tiles
KT = D // 128          # 4 contraction tiles for projections
MCH = 512              # m chunk (q rows per attention chunk)
NCH = S // MCH         # 8 chunks
NT = S // 128          # 32 key tiles
SCALE = 1.0 / np.sqrt(HD)


def _build_body(nc, tc, X, WQ, WK, WV, BQ, BK, BV, WO, OUT, RSCR, ONES, IDENT):
    with (
        tc.tile_pool(name="cons", bufs=1) as cons,
        tc.tile_pool(name="big", bufs=1) as big,
    ):
        ident_r = cons.tile([128, 128], F32R, tag="ident_r")
        nc.sync.dma_start(out=ident_r[:], in_=IDENT[:].bitcast(F32R))
        ident_b = cons.tile([128, 128], BF16, tag="ident_b")
        nc.gpsimd.dma_start(out=ident_b[:], in_=IDENT[:])

        # weight stationaries (f32r via bitcast DMA)
        wq = cons.tile([128, KT, 128], F32R, tag="wq")
        wk = cons.tile([128, KT, 128], F32R, tag="wk")
        wv = cons.tile([128, KT, 128], F32R, tag="wv")
        for w_sb, w_dram in ((wq, WQ), (wk, WK), (wv, WV)):
            nc.sync.dma_start(
                out=w_sb[:],
                in_=w_dram[:].rearrange("(kt p) m -> p kt m", p=128).bitcast(F32R),
            )
        wo_sb = [cons.tile([64, MCH], BF16, tag=f"wo{h}", name=f"wo{h}") for h in range(2)]
        for h in range(2):
            nc.gpsimd.dma_start(out=wo_sb[h][:], in_=WO[h * 64:(h + 1) * 64, :])
        bq = cons.tile([128, 1], F32, tag="bq")
        bk = cons.tile([128, 1], F32, tag="bk")
        bv = cons.tile([128, 1], F32, tag="bv")
        for b_sb, b_dram in ((bq, BQ), (bk, BK), (bv, BV)):
            nc.sync.dma_start(out=b_sb[:], in_=b_dram[:])

        # persistent big tensors
        xT = big.tile([128, KT, S], F32R, tag="xT")       # x^T  (d on partitions)
        qT = big.tile([128, S], BF16, tag="qT")           # heads-stacked q^T
        kTt = big.tile([128, S], BF16, tag="kT")
        v1 = [big.tile([128, NT, HD + 1], BF16, tag=f"v1_{h}", name=f"v1_{h}") for h in range(2)]
        oT = [big.tile([64, S], BF16, tag=f"oT{h}", name=f"oT{h}") for h in range(2)]

        # ---------------- Phase A: x^T via PE transposes (f32r) ------------
        with (
            tc.tile_pool(name="xs_pool", bufs=3) as xs_pool,
            tc.tile_pool(name="vT_pool", bufs=1) as vT_pool,
            tc.tile_pool(name="psA", bufs=4, space="PSUM") as psA,
            tc.tile_pool(name="psB", bufs=3, space="PSUM") as psB,
        ):
            for st in range(ST):
                xs = xs_pool.tile([128, D], F32R, tag="xs")
                nc.sync.dma_start(out=xs[:], in_=X[st * 128:(st + 1) * 128, :].bitcast(F32R))
                for kt in range(KT):
                    pt = psA.tile([128, 128], F32R, tag="tr")
                    nc.tensor.transpose(pt[:], xs[:, kt * 128:(kt + 1) * 128], ident_r[:])
                    nc.vector.tensor_copy(xT[:, kt, st * 128:(st + 1) * 128], pt[:])

            # ---- Phase B+C: projections with v-transposes interleaved -----
            for h in range(2):
                nc.gpsimd.dma_start(
                    out=v1[h][:, :, HD:HD + 1],
                    in_=ONES[:].unsqueeze(2),
                )
            vT = vT_pool.tile([128, S], BF16)
            for ch in range(NCH):
                csl = slice(ch * MCH, (ch + 1) * MCH)
                for w_sb, b_sb, dest in ((wq, bq, qT), (wk, bk, kTt), (wv, bv, vT)):
                    ps = psB.tile([128, MCH], F32, tag="proj", name="ps")
                    for kt in range(KT):
                        nc.tensor.matmul(
                            ps[:], w_sb[:, kt, :], xT[:, kt, csl],
                            start=(kt == 0), stop=(kt == KT - 1),
                        )
                    nc.vector.tensor_scalar_add(dest[:, csl], ps[:], b_sb[:])
                # natural v for the 4 key-tiles this chunk covers
                for h in range(2):
                    for nt in range(4 * ch, 4 * ch + 4):
                        pt = psA.tile([128, HD], BF16, tag="tr", name="pt")
                        nc.tensor.transpose(
                            pt[:],
                            vT[h * 64:(h + 1) * 64, nt * 128:(nt + 1) * 128],
                            ident_b[h * 64:(h + 1) * 64, h * 64:(h + 1) * 64],
                        )
                        nc.vector.tensor_copy(v1[h][:, nt, 0:HD], pt[:])

        # ---------------- Phase D: attention -------------------------------
        with (
            tc.tile_pool(name="esP", bufs=4) as esP,
            tc.tile_pool(name="nrm", bufs=4) as nrm,
            tc.tile_pool(name="psS", bufs=2, space="PSUM") as psS_pool,
            tc.tile_pool(name="psO", bufs=4, space="PSUM") as psO_pool,
        ):
            for ch in range(NCH):
                mslice = slice(ch * MCH, (ch + 1) * MCH)
                psO = [psO_pool.tile([HD + 1, MCH], F32, tag="psO", name="psO") for _ in range(2)]

                def emit_pv(np_, eS_pair):
                    for h in range(2):
                        for j in range(2):
                            nt = np_ * 2 + j
                            nc.tensor.matmul(
                                psO[h][:], v1[h][:, nt, :],
                                eS_pair[h][:, j * 512:(j + 1) * 512],
                                start=(nt == 0), stop=(nt == NT - 1),
                                skip_group_check=True,
                            )

                prev = None
                for npair in range(NT // 2):
                    psS = [psS_pool.tile([128, 1024], F32, tag="psS", name="psS") for _ in range(2)]
                    for j in range(2):
                        nt = npair * 2 + j
                        for h in range(2):
                            nc.tensor.matmul(
                                psS[h][:, j * 512:(j + 1) * 512],
                                kTt[h * 64:(h + 1) * 64, nt * 128:(nt + 1) * 128],
                                qT[h * 64:(h + 1) * 64, mslice],
                                start=True, stop=True,
                            )
                    eS_pair = [esP.tile([128, 1024], BF16, tag=f"eS{h}", name=f"eS{h}") for h in range(2)]
                    for h in range(2):
                        nc.scalar.activation(eS_pair[h][:], psS[h][:], AF.Exp, scale=float(SCALE))
                    if prev is not None:
                        emit_pv(*prev)
                    prev = (npair, eS_pair)
                emit_pv(*prev)

                # normalize: oT[h] = psO[rows 0:64] * (1 / psO[row 64]) (bcast)
                for h in range(2):
                    rd = nrm.tile([1, MCH], F32, tag="rd")
                    nc.vector.reciprocal(rd[:], psO[h][HD:HD + 1, :])
                    nc.sync.dma_start(out=RSCR[ch, h, :].unsqueeze(0), in_=rd[:])
                    rb = nrm.tile([64, MCH], F32, tag="rb")
                    src = RSCR[ch, h, :]
                    bcast = bass.AP(tensor=src.tensor, offset=src.offset,
                                    ap=[[0, 64]] + list(src.ap))
                    nc.gpsimd.dma_start(out=rb[:], in_=bcast)
                    nc.vector.tensor_tensor(
                        out=oT[h][:, mslice], in0=psO[h][0:HD, :], in1=rb[:],
                        op=mybir.AluOpType.mult,
                    )

        # ---------------- Phase E: output projection -----------------------
        with (
            tc.tile_pool(name="outP", bufs=3) as outP,
            tc.tile_pool(name="psE", bufs=3, space="PSUM") as psE,
        ):
            for mt in range(ST):
                ps = psE.tile([128, MCH], F32, tag="psE")
                for h in range(2):
                    nc.tensor.matmul(
                        ps[:], oT[h][:, mt * 128:(mt + 1) * 128], wo_sb[h][:],
                        start=(h == 0), stop=(h == 1),
                    )
                ob = outP.tile([128, MCH], F32, tag="ob")
                nc.vector.tensor_copy(ob[:], ps[:])
                nc.sync.dma_start(out=OUT[mt * 128:(mt + 1) * 128, :], in_=ob[:])


def build_kernel():
    nc = bacc.Bacc("TRN2", target_bir_lowering=False)
    with tile.TileContext(nc) as tc:
        with tc.tile_pool(name="dram", bufs=1, space="DRAM") as dram:
            X = dram.tile([S, D], F32, kind="ExternalInput")
            WQ = dram.tile([D, 128], F32, kind="ExternalInput")
            WK = dram.tile([D, 128], F32, kind="ExternalInput")
            WV = dram.tile([D, 128], F32, kind="ExternalInput")
            BQ = dram.tile([128, 1], F32, kind="ExternalInput")
            BK = dram.tile([128, 1], F32, kind="ExternalInput")
            BV = dram.tile([128, 1], F32, kind="ExternalInput")
            WO = dram.tile([128, D], F32, kind="ExternalInput")
            OUT = dram.tile([S, D], F32, kind="ExternalOutput")
            RSCR = dram.tile([NCH, 2, MCH], F32, kind="Internal")
            ONES = dram.tile([128, NT], F32, kind="ExternalInput")
            IDENT = dram.tile([128, 128], F32, kind="ExternalInput")
            _build_body(nc, tc, X, WQ, WK, WV, BQ, BK, BV, WO, OUT, RSCR, ONES, IDENT)
    nc.compile()
    names = dict(X=X.name, WQ=WQ.name, WK=WK.name, WV=WV.name, BQ=BQ.name,
                 BK=BK.name, BV=BV.name, WO=WO.name, OUT=OUT.name,
                 ONES=ONES.name, IDENT=IDENT.name)
    return nc, names


_CACHE = {}


def _get_kernel():
    if "k" not in _CACHE:
        _CACHE["k"] = build_kernel()
    return _CACHE["k"]


def make_in_maps(hidden_states, Wq, bq, Wk, bk, Wv, bv, Wo, bo, names):
    x = np.asarray(hidden_states, dtype=np.float32).reshape(B, S, D)
    ones = np.ones((128, NT), dtype=np.float32)
    ident = np.eye(128, dtype=np.float32)
    in_maps = []
    for c in range(NCORES):
        b, hp = c // 4, c % 4
        sl = slice(hp * 128, (hp + 1) * 128)
        in_maps.append({
            names["X"]: np.ascontiguousarray(x[b]),
            names["WQ"]: np.ascontiguousarray(np.asarray(Wq, np.float32)[:, sl]),
            names["WK"]: np.ascontiguousarray(np.asarray(Wk, np.float32)[:, sl]),
            names["WV"]: np.ascontiguousarray(np.asarray(Wv, np.float32)[:, sl]),
            names["BQ"]: np.ascontiguousarray(np.asarray(bq, np.float32)[sl].reshape(128, 1)),
            names["BK"]: np.ascontiguousarray(np.asarray(bk, np.float32)[sl].reshape(128, 1)),
            names["BV"]: np.ascontiguousarray(np.asarray(bv, np.float32)[sl].reshape(128, 1)),
            names["WO"]: np.ascontiguousarray(np.asarray(Wo, np.float32)[sl, :]),
            names["ONES"]: ones,
            names["IDENT"]: ident,
        })
    return in_maps


def combine_outputs(results, names, bo):
    out = np.zeros((B, S, D), dtype=np.float64)
    for c in range(NCORES):
        out[c // 4] += results[c][names["OUT"]].astype(np.float64)
    out += np.asarray(bo, np.float64)
    return out.astype(np.float32)


def run(hidden_states, Wq, bq, Wk, bk, Wv, bv, Wo, bo, trace=False, **spmd_kwargs):
    nc, names = _get_kernel()
    in_maps = make_in_maps(hidden_states, Wq, bq, Wk, bk, Wv, bv, Wo, bo, names)
    res = bass_utils.run_bass_kernel_spmd(
        nc, in_maps, core_ids=list(range(NCORES)), trace=trace, **spmd_kwargs
    )
    out = combine_outputs(res.results, names, bo)
    return out, res


def kernel(hidden_states, Wq, bq, Wk, bk, Wv, bv, Wo, bo):
    out, _ = run(hidden_states, Wq, bq, Wk, bk, Wv, bv, Wo, bo, trace=False)
    return out
